# revision 9
# baseline (speedup 1.0000x reference)
"""Causal self-attention Trainium2 Bass kernel (v2, bf16 pipeline).

Problem: nn_CausalSelfAttention (B=2, L=2048, D=1024, H=16 heads, Khd=64).

Sharding (8 cores): data-parallel over B (2 way) x tensor-parallel over
heads (4 way, 4 heads/core).  Each core computes
  qkv_local = x_b @ W_attn_local.T          (c_attn column-sharded)
  attn_local = causal_attention(q,k,v)      (4 heads)
  y_partial  = attn_local @ W_proj_local.T  (c_proj row-sharded)
and the host sums the 4 partials per batch (the row-parallel unshard).

v2 layout/schedule choices:
  - every matmul operand is bf16 (same PE col/cycle rate as f32r, but
    half the DMA/SBUF traffic, FWL weight loads, 2x DVE on sbuf ops);
    all accumulation stays fp32 in PSUM.  End-to-end absmax-rel error
    vs fp64 is ~4e-3 (gate 2e-2).
  - qT,kT computed feature-major [64*4, L] directly (weights as lhsT),
    so attention score matmuls need no transposes; the per-plane layout
    (head pair at partitions 0-63 / 64-127) makes the two heads' score
    matmuls row-tiled (tile_position (0,0)/(64,0)) -> they run
    concurrently on the PE array (K=64 each).
  - scores computed transposed: scT[kcol, qrow] = kT.T-block @ qT-block.
  - softmax denominator via an extra ones-row appended to V (so the
    PV^T matmul also produces the row sums); 1/denominator is spread
    across partitions with a GpSimd partition_broadcast, multiply on DVE.
  - QKV projection, V, and output projection are decomposed into small
    "filler units" that are interleaved into the attention rounds
    between the score matmuls and the (exp-dependent) PV matmuls, so
    the in-order PE stream never stalls waiting on ScalarE's exp.
"""

import math

import numpy as np

B, L, D, H = 2, 2048, 1024, 16
KHD = D // H  # 64 head dim
NCORES = 8
HPC = 4  # heads per core
FQK = 2 * HPC * KHD  # 512 q+k local features
FV = HPC * KHD  # 256 v local features
FQKV = FQK + FV  # 768
DK = D // 128  # 8 contraction chunks
LC = L // 128  # 16 row chunks
NJ = L // 512  # 4 qrow blocks
SCALE = 1.0 / math.sqrt(KHD)

_CACHE = {}


def _build(has_bqk: bool, has_bv: bool, has_bp: bool, reps: int = 1):
    import concourse.bass as bass
    import concourse.mybir as mybir
    import concourse.tile as tile
    from concourse import bacc

    f32 = mybir.dt.float32
    bf16 = mybir.dt.bfloat16

    nc = bacc.Bacc(None, target_bir_lowering=False)
    xT_d = nc.declare_dram_parameter("xT", [D, L], bf16, isOutput=False)
    wqkvT_d = nc.declare_dram_parameter("wqkvT", [D, FQKV], bf16, isOutput=False)
    wpT_d = nc.declare_dram_parameter("wpT", [FV, D], bf16, isOutput=False)
    tri_d = nc.declare_dram_parameter("tri", [128, 128], bf16, isOutput=False)
    ones2d_d = nc.declare_dram_parameter("ones2d", [128, LC * HPC], bf16, isOutput=False)
    onesrow_d = nc.declare_dram_parameter("onesrow", [1, 128], bf16, isOutput=False)
    if has_bqk:
        bqk_d = nc.declare_dram_parameter("bqk", [128, FQK // 128], f32, isOutput=False)
    if has_bv:
        bv_d = nc.declare_dram_parameter("bv", [1, FV], bf16, isOutput=False)
    if has_bp:
        bp_d = nc.declare_dram_parameter("bp", [1, D], bf16, isOutput=False)
    y_d = nc.declare_dram_parameter("y", [L, D], bf16, isOutput=True)

    with nc.allow_low_precision(reason="bf16 matmul pipeline"), tile.TileContext(nc) as tc:
        with (
            tc.tile_pool(name="persist", bufs=1) as persist,
            tc.tile_pool(name="work", bufs=3) as work,
            tc.tile_pool(name="small", bufs=2) as small,
            tc.tile_pool(name="ps_sc", bufs=2, space="PSUM") as ps_sc,
            tc.tile_pool(name="ps_out", bufs=2, space="PSUM") as ps_out,
            tc.tile_pool(name="ps_fill", bufs=2, space="PSUM") as ps_fill,
        ):
            for _rep in range(reps):
                # ---- persistent SBUF tensors ----
                xT_sbs = [persist.tile([128, L], bf16, name=f"xT{k}", tag=f"xT{k}") for k in range(DK)]
                wqkvT_sbs = [
                    persist.tile([128, FQKV], bf16, name=f"wqkvT{k}", tag=f"wqkvT{k}") for k in range(DK)
                ]
                # per-plane q/k (plane p holds heads 2p, 2p+1)
                qT_p = [persist.tile([128, L], bf16, name=f"qT{p}", tag=f"qT{p}") for p in range(2)]
                kT_p = [persist.tile([128, L], bf16, name=f"kT{p}", tag=f"kT{p}") for p in range(2)]
                v_augs = [
                    persist.tile([128, HPC, KHD + 1], bf16, name=f"vaug{lc}", tag=f"vaug{lc}")
                    for lc in range(LC)
                ]
                attnT_js = [
                    persist.tile([128, 2, 512], bf16, name=f"attnT{j}", tag=f"attnT{j}")
                    for j in range(NJ)
                ]
                wpT_sbs = [persist.tile([128, D], bf16, name=f"wpT{k}", tag=f"wpT{k}") for k in range(2)]
                tri_sb = persist.tile([128, 128], bf16)
                ones_row = persist.tile([1, 128], bf16)

                # ---- warm the ScalarE exp table while DMA runs: the first
                # real exp would otherwise pay the ~2.7us ACT_TABLE_LOAD on
                # the critical path.
                warm_in = small.tile([1, 8], f32, tag="warm_i", name="warm_i")
                warm_out = small.tile([1, 8], f32, tag="warm_o", name="warm_o")
                nc.vector.memset(warm_in, 0.0)
                nc.scalar.activation(
                    warm_out, warm_in, mybir.ActivationFunctionType.Exp
                )

                # ---- input DMA: interleave weight chunks with the first
                # x q-block so the first QKV accumulation chases the DMA
                # stream; remaining x pieces in q-block order.
                for k in range(DK):
                    nc.sync.dma_start(out=wqkvT_sbs[k], in_=wqkvT_d[k * 128:(k + 1) * 128, :])
                    eng = nc.sync if k % 2 == 0 else nc.gpsimd
                    eng.dma_start(
                        out=xT_sbs[k][:, 0:512],
                        in_=xT_d[k * 128:(k + 1) * 128, 0:512],
                    )
                nc.sync.dma_start(out=tri_sb, in_=tri_d[:])
                nc.sync.dma_start(out=ones_row, in_=onesrow_d[:])
                for lc in range(LC):
                    nc.sync.dma_start(
                        out=v_augs[lc][:, :, KHD],
                        in_=ones2d_d[:, lc * HPC:(lc + 1) * HPC],
                    )
                if has_bqk:
                    bqk_sb = persist.tile([128, FQK // 128], f32)
                    nc.sync.dma_start(out=bqk_sb, in_=bqk_d[:])
                if has_bv:
                    bv_sb = persist.tile([1, FV], bf16)
                    nc.sync.dma_start(out=bv_sb, in_=bv_d[:])
                if has_bp:
                    bp_sb = persist.tile([1, D], bf16)
                    nc.sync.dma_start(out=bp_sb, in_=bp_d[:])
                for i in range(1, NJ):
                    for k in range(DK):
                        eng = nc.sync if k % 2 == 0 else nc.gpsimd
                        eng.dma_start(
                            out=xT_sbs[k][:, i * 512:(i + 1) * 512],
                            in_=xT_d[k * 128:(k + 1) * 128, i * 512:(i + 1) * 512],
                        )
                for kc in range(2):
                    nc.sync.dma_start(
                        out=wpT_sbs[kc], in_=wpT_d[kc * 128:(kc + 1) * 128, :]
                    )

                # ---- filler units: small PE work packets interleaved into
                # the attention rounds so PE never idles on ScalarE ----

                def emit_qk_unit(m, i):
                    # feature chunk m (0,1: q planes; 2,3: k planes),
                    # q-columns i*512..i*512+511
                    dst = qT_p[m] if m < 2 else kT_p[m - 2]
                    ps = ps_fill.tile([128, 512], f32, tag="fill", name="psqk")
                    for k in range(DK):
                        nc.tensor.matmul(
                            ps,
                            wqkvT_sbs[k][:, m * 128:(m + 1) * 128],
                            xT_sbs[k][:, i * 512:(i + 1) * 512],
                            start=(k == 0),
                            stop=(k == DK - 1),
                        )
                    sl = slice(i * 512, (i + 1) * 512)
                    if has_bqk:
                        nc.scalar.activation(
                            dst[:, sl], ps,
                            mybir.ActivationFunctionType.Copy,
                            bias=bqk_sb[:, m:m + 1],
                        )
                    else:
                        nc.vector.tensor_copy(out=dst[:, sl], in_=ps)

                def emit_v_unit(lc):
                    ps = ps_fill.tile([128, 512], f32, tag="fill", name="psv")
                    for k in range(DK):
                        nc.tensor.matmul(
                            ps[:, 0:FV],
                            xT_sbs[k][:, lc * 128:(lc + 1) * 128],
                            wqkvT_sbs[k][:, FQK:FQKV],
                            start=(k == 0),
                            stop=(k == DK - 1) and not has_bv,
                        )
                    if has_bv:
                        nc.tensor.matmul(
                            ps[:, 0:FV], ones_row[0:1, :], bv_sb,
                            start=False, stop=True,
                        )
                    nc.vector.tensor_copy(
                        out=v_augs[lc][:, :, 0:KHD],
                        in_=ps[:, 0:FV].rearrange("p (h k) -> p h k", h=HPC),
                    )

                def emit_proj_unit(j, lq, half):
                    # projection of row chunk 4j+lq, output cols half*512..
                    lc = 4 * j + lq
                    sl = slice(half * 512, (half + 1) * 512)
                    psy = ps_fill.tile([128, 512], f32, tag="fill", name="psy")
                    for kc in range(2):
                        nc.tensor.matmul(
                            psy,
                            attnT_js[j][:, kc, lq * 128:(lq + 1) * 128],
                            wpT_sbs[kc][:, sl],
                            start=(kc == 0),
                            stop=(kc == 1) and not has_bp,
                        )
                    if has_bp:
                        nc.tensor.matmul(
                            psy, ones_row[0:1, :], bp_sb[0:1, sl],
                            start=False, stop=True,
                        )
                    ysb = work.tile([128, 512], bf16, tag="ysb", name="ysb", bufs=4)
                    if j == NJ - 1 and lq >= 2:
                        # tail blocks: ScalarE is idle after the last exp
                        nc.scalar.activation(
                            ysb, psy, mybir.ActivationFunctionType.Copy
                        )
                    else:
                        nc.vector.tensor_copy(out=ysb, in_=psy)
                    nc.sync.dma_start(out=y_d[lc * 128:(lc + 1) * 128, sl], in_=ysb)

                units = []  # (cost_ns, fn)
                QK_NS, V_NS, PJ_NS = 1707.0, 853.0, 427.0
                prefix = {}
                for i in range(NJ):
                    units.append((QK_NS, (lambda m=0, i=i: emit_qk_unit(m, i))))
                    units.append((QK_NS, (lambda m=2, i=i: emit_qk_unit(m, i))))
                    if i == 0:
                        # V0-3 are injected inside att(0,0) (between its exps
                        # and the PV flushes) so the first exp starts sooner
                        prefix[(0, 0)] = len(units)
                    else:
                        for lc in range(4 * i, 4 * i + 4):
                            units.append((V_NS, (lambda lc=lc: emit_v_unit(lc))))
                        prefix[(i, 0)] = len(units)
                    units.append((QK_NS, (lambda m=1, i=i: emit_qk_unit(m, i))))
                    units.append((QK_NS, (lambda m=3, i=i: emit_qk_unit(m, i))))
                    prefix[(i, 2)] = len(units)

                def prefix_for(j, h0):
                    return prefix[(j, h0)]

                state = {"ptr": 0, "debt": 0.0}

                def drain_until(idx):
                    while state["ptr"] < idx:
                        cost, fn = units[state["ptr"]]
                        state["ptr"] += 1
                        fn()

                def drain_ns(budget):
                    state["debt"] += budget
                    while state["ptr"] < len(units) and state["debt"] > 0:
                        cost, fn = units[state["ptr"]]
                        state["ptr"] += 1
                        state["debt"] -= cost
                        fn()

                # ---- attention ----
                def emit_att_pair(j, h0, inject=()):
                    # two heads (same q/k plane) processed in lockstep, with the
                    # PV matmuls software-pipelined one round behind the scores
                    # so PE never waits on ScalarE's exp; filler units drained
                    # between the scores and the exp-gated PV flush.
                    pl = h0 // 2
                    pos = [0, 64]
                    heads = [h0, h0 + 1]
                    outTs = [
                        ps_out.tile([KHD + 1, 512], f32, tag="outT", name="outT")
                        for _ in range(2)
                    ]
                    qrs = slice(j * 512, (j + 1) * 512)
                    last_c = 4 * j + 3
                    rounds = [("below", cp) for cp in range(0, 4 * j, 2)]
                    rounds += [("diag", 0), ("diag", 2)]
                    pending = []
                    inject = list(inject)

                    def flush_pending():
                        if inject and pending:
                            # emit the injected units (V chunks) this flush's
                            # PV matmuls depend on
                            need = max(c for _, parts, _ in pending for c, _, _ in parts) + 1
                            while inject and inject[0][0] < need:
                                _, fn = inject.pop(0)
                                fn()
                        for hh, parts, ex in pending:
                            for c, exsl, n0 in parts:
                                nc.tensor.matmul(
                                    outTs[hh][:, n0:512],
                                    v_augs[c][:, heads[hh], :],
                                    ex[:, exsl],
                                    start=(c == 0),
                                    stop=(c == last_c),
                                )
                        pending.clear()

                    for kind, arg in rounds:
                        new_pending = []
                        for hh in range(2):
                            po = pos[hh]
                            sc = ps_sc.tile([128, 1024], f32, tag="sc", name="sc")
                            if kind == "below":
                                cp = arg
                                for half in range(2):
                                    c = cp + half
                                    nc.tensor.matmul(
                                        sc[:, half * 512:(half + 1) * 512],
                                        kT_p[pl][po:po + 64, c * 128:(c + 1) * 128],
                                        qT_p[pl][po:po + 64, qrs],
                                        start=True,
                                        stop=True,
                                    )
                                ex = work.tile([128, 1024], bf16, tag="expT", name="ex", bufs=6)
                                nc.scalar.activation(
                                    ex, sc,
                                    mybir.ActivationFunctionType.Exp, scale=SCALE,
                                )
                                parts = [
                                    (cp, slice(0, 512), 0),
                                    (cp + 1, slice(512, 1024), 0),
                                ]
                            else:
                                i0 = arg
                                ws = [512 - 128 * (i0 + di) for di in range(2)]
                                offs = [0, ws[0]]
                                wtot = ws[0] + ws[1]
                                for di in range(2):
                                    c = 4 * j + i0 + di
                                    n0 = 128 * (i0 + di)
                                    nc.tensor.matmul(
                                        sc[:, offs[di]:offs[di] + ws[di]],
                                        kT_p[pl][po:po + 64, c * 128:(c + 1) * 128],
                                        qT_p[pl][po:po + 64, j * 512 + n0:(j + 1) * 512],
                                        start=True,
                                        stop=True,
                                    )
                                ex = work.tile([128, 1024], bf16, tag="expT", name="ex", bufs=6)
                                nc.scalar.activation(
                                    ex[:, 0:wtot], sc[:, 0:wtot],
                                    mybir.ActivationFunctionType.Exp, scale=SCALE,
                                )
                                for di in range(2):
                                    nc.vector.tensor_mul(
                                        ex[:, offs[di]:offs[di] + 128],
                                        ex[:, offs[di]:offs[di] + 128],
                                        tri_sb,
                                    )
                                parts = [
                                    (4 * j + i0, slice(0, ws[0]), 128 * i0),
                                    (
                                        4 * j + i0 + 1,
                                        slice(offs[1], offs[1] + ws[1]),
                                        128 * (i0 + 1),
                                    ),
                                ]
                            new_pending.append((hh, parts, ex))
                        # filler between the scores and the exp-gated PV flush
                        if kind == "below":
                            drain_ns(800.0)
                        elif arg == 0:
                            drain_ns(650.0)
                        else:
                            drain_ns(120.0)
                        flush_pending()
                        pending.extend(new_pending)
                    flush_pending()
                    # normalize: attnT[f, qrow] = outT[f, qrow] / outT[64, qrow]
                    for hh in range(2):
                        po = pos[hh]
                        recip = small.tile([1, 512], bf16, tag="recip", name="recip")
                        nc.vector.reciprocal(recip, outTs[hh][KHD:KHD + 1, :])
                        bc_sb = small.tile([64, 512], bf16, tag="bcsb", name="bc_sb")
                        nc.gpsimd.partition_broadcast(bc_sb, recip)
                        nc.vector.tensor_mul(
                            attnT_js[j][po:po + 64, pl, :], outTs[hh][0:KHD, :], bc_sb
                        )

                # ---- emission schedule ----
                for j in range(NJ):
                    for h0 in (0, 2):
                        drain_until(prefix_for(j, h0))
                        inj = ()
                        if j == 0 and h0 == 0:
                            inj = [(lc, (lambda lc=lc: emit_v_unit(lc)))
                                   for lc in range(4)]
                        emit_att_pair(j, h0, inj)
                    # projection units for this j become available now
                    for lq in range(4):
                        for half in range(2):
                            units.append(
                                (PJ_NS, (lambda j=j, lq=lq, half=half:
                                         emit_proj_unit(j, lq, half)))
                            )
                drain_until(len(units))

    nc.compile()
    return nc


def kernel(input_BLD, W_attn, b_attn, W_proj, b_proj):
    import ml_dtypes

    bf16 = ml_dtypes.bfloat16
    input_BLD = np.asarray(input_BLD, dtype=np.float32)
    W_attn = np.asarray(W_attn, dtype=np.float32)
    b_attn = np.asarray(b_attn, dtype=np.float32)
    W_proj = np.asarray(W_proj, dtype=np.float32)
    b_proj = np.asarray(b_proj, dtype=np.float32)

    has_bqk = bool(np.any(b_attn[: 2 * D]))
    has_bv = bool(np.any(b_attn[2 * D:]))
    has_bp = bool(np.any(b_proj))

    key = (has_bqk, has_bv, has_bp)
    if key not in _CACHE:
        _CACHE[key] = _build(*key)
    nc = _CACHE[key]

    tri = (np.arange(128)[None, :] >= np.arange(128)[:, None]).astype(bf16)
    in_maps = []
    for c in range(NCORES):
        b, t = divmod(c, 4)
        hs = t * HPC * KHD  # feature offset of this core's heads
        w_loc = np.concatenate(
            [
                W_attn[hs:hs + FV],  # q rows
                W_attn[D + hs:D + hs + FV],  # k rows
                W_attn[2 * D + hs:2 * D + hs + FV],  # v rows
            ],
            axis=0,
        )  # [768, 1024]
        m = {
            "xT": np.ascontiguousarray(input_BLD[b].T).astype(bf16),
            "wqkvT": np.ascontiguousarray(w_loc.T).astype(bf16),
            "wpT": np.ascontiguousarray(W_proj[:, hs:hs + FV].T).astype(bf16),
            "tri": tri,
            "ones2d": np.ones((128, LC * HPC), bf16),
            "onesrow": np.ones((1, 128), bf16),
        }
        if has_bqk:
            bqk = np.concatenate([b_attn[hs:hs + FV], b_attn[D + hs:D + hs + FV]])
            m["bqk"] = np.ascontiguousarray(bqk.reshape(FQK // 128, 128).T)
        if has_bv:
            m["bv"] = b_attn[2 * D + hs:2 * D + hs + FV][None, :].astype(bf16)
        if has_bp:
            m["bp"] = (b_proj / 4.0)[None, :].astype(bf16)
        in_maps.append(m)

    from concourse.bass_utils import run_bass_kernel_spmd

    globals()["_last_in_maps"] = in_maps
    res = run_bass_kernel_spmd(nc, in_maps, list(range(NCORES)))
    globals()["_LAST_RESULTS"] = res
    out = np.empty((B, L, D), dtype=np.float32)
    for b in range(B):
        acc = res.results[4 * b]["y"].astype(np.float32)
        for t in range(1, 4):
            acc = acc + res.results[4 * b + t]["y"].astype(np.float32)
        out[b] = acc
    return out


# revision 11
# speedup vs baseline: 1.4987x; 1.4987x over previous
"""Causal self-attention Trainium2 Bass kernel (v4, bf16 + batched DMA).

Problem: nn_CausalSelfAttention (B=2, L=2048, D=1024, H=16 heads, Khd=64).

Sharding (8 cores): data-parallel over B (2 way) x tensor-parallel over
heads (4 way, 4 heads/core).  Each core computes
  qkv_local = x_b @ W_attn_local.T          (c_attn column-sharded)
  attn_local = causal_attention(q,k,v)      (4 heads)
  y_partial  = attn_local @ W_proj_local.T  (c_proj row-sharded)
and the host sums the 4 partials per batch (the row-parallel unshard).

Kernel design:
  - every matmul operand is bf16 (same PE col/cycle rate as f32r, half
    the DMA/SBUF traffic, FWL weight loads); accumulation fp32 in PSUM.
    End-to-end absmax-rel error vs fp64 is ~4e-3 (gate 2e-2).
  - qT,kT computed feature-major [64*4, L]; the per-plane layout (head
    pair at partitions 0-63 / 64-127) makes the two heads' score
    matmuls row-tiled (tile_position (0,0)/(64,0)) -> concurrent.
  - softmax denominator via an extra ones-row appended to V; the
    reciprocal is partition-broadcast on GpSimd, multiply on DVE.
  - QKV/V/proj work is decomposed into small units drained into the
    attention rounds between scores and the exp-gated PV flush, so the
    in-order PE stream never stalls on ScalarE.
  - DMA is batched into few multi-dim transfers (the serial per-DMA
    dispatch on the issuing sequencer was costing ~300ns each) and
    spread across the SP/Activation HWDGE rings + gpsimd SWDGE; the
    w/x chunks for the first q-block stay fine-grained so the first
    QKV accumulation chases the DMA stream.
"""

import math

import numpy as np

B, L, D, H = 2, 2048, 1024, 16
KHD = D // H  # 64 head dim
NCORES = 8
HPC = 4  # heads per core
FQK = 2 * HPC * KHD  # 512 q+k local features
FV = HPC * KHD  # 256 v local features
FQKV = FQK + FV  # 768
DK = D // 128  # 8 contraction chunks
LC = L // 128  # 16 row chunks
NJ = L // 512  # 4 qrow blocks
SCALE = 1.0 / math.sqrt(KHD)

_CACHE = {}


def _build(has_bqk: bool, has_bv: bool, has_bp: bool, reps: int = 1):
    import concourse.bass as bass
    import concourse.mybir as mybir
    import concourse.tile as tile
    from concourse import bacc

    f32 = mybir.dt.float32
    bf16 = mybir.dt.bfloat16
    any_bias = has_bv or has_bp

    nc = bacc.Bacc(None, target_bir_lowering=False)
    xT_d = nc.declare_dram_parameter("xT", [D, L], bf16, isOutput=False)
    wqkvT_d = nc.declare_dram_parameter("wqkvT", [D, FQKV], bf16, isOutput=False)
    wpT_d = nc.declare_dram_parameter("wpT", [FV, D], bf16, isOutput=False)
    tri_d = nc.declare_dram_parameter("tri", [128, 128], bf16, isOutput=False)
    if any_bias:
        onesrow_d = nc.declare_dram_parameter("onesrow", [1, 128], bf16, isOutput=False)
    if has_bqk:
        bqk_d = nc.declare_dram_parameter("bqk", [128, FQK // 128], f32, isOutput=False)
    if has_bv:
        bv_d = nc.declare_dram_parameter("bv", [1, FV], bf16, isOutput=False)
    if has_bp:
        bp_d = nc.declare_dram_parameter("bp", [1, D], bf16, isOutput=False)
    y_d = nc.declare_dram_parameter("y", [L, D], bf16, isOutput=True)

    with nc.allow_low_precision(reason="bf16 matmul pipeline"), tile.TileContext(nc) as tc:
        with (
            tc.tile_pool(name="persist", bufs=1) as persist,
            tc.tile_pool(name="work", bufs=3) as work,
            tc.tile_pool(name="small", bufs=2) as small,
            tc.tile_pool(name="ps_sc", bufs=2, space="PSUM") as ps_sc,
            tc.tile_pool(name="ps_out", bufs=2, space="PSUM") as ps_out,
            tc.tile_pool(name="ps_fill", bufs=2, space="PSUM") as ps_fill,
        ):
            for _rep in range(reps):
                # ---- persistent SBUF tensors ----
                x_all = persist.tile([128, DK, L], bf16, name="x_all", tag="x_all")
                wq_all = persist.tile([128, DK, FQKV], bf16, name="wq_all", tag="wq_all")
                qT_p = [persist.tile([128, L], bf16, name=f"qT{p}", tag=f"qT{p}") for p in range(2)]
                kT_p = [persist.tile([128, L], bf16, name=f"kT{p}", tag=f"kT{p}") for p in range(2)]
                vaug = persist.tile([128, LC, HPC, KHD + 1], bf16, name="vaug", tag="vaug")
                attnT_js = [
                    persist.tile([128, 2, 512], bf16, name=f"attnT{j}", tag=f"attnT{j}")
                    for j in range(NJ)
                ]
                wp_all = persist.tile([128, 2, D], bf16, name="wp_all", tag="wp_all")
                tri_sb = persist.tile([128, 128], bf16)

                # ---- warm the ScalarE exp table while DMA runs ----
                warm_in = small.tile([1, 8], f32, tag="warm_i", name="warm_i")
                warm_out = small.tile([1, 8], f32, tag="warm_o", name="warm_o")
                nc.vector.memset(warm_in, 0.0)
                nc.scalar.activation(
                    warm_out, warm_in, mybir.ActivationFunctionType.Exp
                )
                # softmax-denominator ones row inside the V tile: memset the
                # whole tile (packed); the V copies later overwrite cols 0:64
                nc.vector.memset(vaug, 1.0)

                # ---- input DMA ----
                # first q-block (i=0) w/x chunks stay fine-grained across the
                # two HWDGE rings so the first QKV accumulation can chase
                for k in range(DK):
                    eng = nc.sync if k % 2 == 0 else nc.scalar
                    eng.dma_start(
                        out=wq_all[:, k, :], in_=wqkvT_d[k * 128:(k + 1) * 128, :]
                    )
                    eng.dma_start(
                        out=x_all[:, k, 0:512],
                        in_=xT_d[k * 128:(k + 1) * 128, 0:512],
                    )
                # remaining q-blocks: one batched transfer each
                x_engs = [nc.sync, nc.scalar, nc.gpsimd]
                for i in range(1, NJ):
                    sl = slice(i * 512, (i + 1) * 512)
                    x_engs[i - 1].dma_start(
                        out=x_all[:, :, sl],
                        in_=xT_d[:, sl].rearrange("(k p) c -> p k c", k=DK),
                    )
                nc.gpsimd.dma_start(out=tri_sb, in_=tri_d[:])
                nc.scalar.dma_start(
                    out=wp_all, in_=wpT_d[:].rearrange("(k p) c -> p k c", k=2)
                )
                if any_bias:
                    ones_row = persist.tile([1, 128], bf16)
                    nc.gpsimd.dma_start(out=ones_row, in_=onesrow_d[:])
                if has_bqk:
                    bqk_sb = persist.tile([128, FQK // 128], f32)
                    nc.gpsimd.dma_start(out=bqk_sb, in_=bqk_d[:])
                if has_bv:
                    bv_sb = persist.tile([1, FV], bf16)
                    nc.gpsimd.dma_start(out=bv_sb, in_=bv_d[:])
                if has_bp:
                    bp_sb = persist.tile([1, D], bf16)
                    nc.gpsimd.dma_start(out=bp_sb, in_=bp_d[:])

                # ---- filler units ----
                def emit_qk_unit(m, i):
                    # feature chunk m (0,1: q planes; 2,3: k planes),
                    # q-columns i*512..i*512+511
                    dst = qT_p[m] if m < 2 else kT_p[m - 2]
                    ps = ps_fill.tile([128, 512], f32, tag="fill", name="psqk")
                    for k in range(DK):
                        nc.tensor.matmul(
                            ps,
                            wq_all[:, k, m * 128:(m + 1) * 128],
                            x_all[:, k, i * 512:(i + 1) * 512],
                            start=(k == 0),
                            stop=(k == DK - 1),
                        )
                    sl = slice(i * 512, (i + 1) * 512)
                    if has_bqk:
                        nc.scalar.activation(
                            dst[:, sl], ps,
                            mybir.ActivationFunctionType.Copy,
                            bias=bqk_sb[:, m:m + 1],
                        )
                    else:
                        nc.vector.tensor_copy(out=dst[:, sl], in_=ps)

                def emit_v_unit(lc):
                    ps = ps_fill.tile([128, 512], f32, tag="fill", name="psv")
                    for k in range(DK):
                        nc.tensor.matmul(
                            ps[:, 0:FV],
                            x_all[:, k, lc * 128:(lc + 1) * 128],
                            wq_all[:, k, FQK:FQKV],
                            start=(k == 0),
                            stop=(k == DK - 1) and not has_bv,
                        )
                    if has_bv:
                        nc.tensor.matmul(
                            ps[:, 0:FV], ones_row[0:1, :], bv_sb,
                            start=False, stop=True,
                        )
                    nc.vector.tensor_copy(
                        out=vaug[:, lc, :, 0:KHD],
                        in_=ps[:, 0:FV].rearrange("p (h k) -> p h k", h=HPC),
                    )

                ysb_tiles = {}

                def emit_proj_unit(j, lq, half):
                    # projection of row chunk 4j+lq, output cols half*512..
                    lc = 4 * j + lq
                    sl = slice(half * 512, (half + 1) * 512)
                    psy = ps_fill.tile([128, 512], f32, tag="fill", name="psy")
                    for kc in range(2):
                        nc.tensor.matmul(
                            psy,
                            attnT_js[j][:, kc, lq * 128:(lq + 1) * 128],
                            wp_all[:, kc, sl],
                            start=(kc == 0),
                            stop=(kc == 1) and not has_bp,
                        )
                    if has_bp:
                        nc.tensor.matmul(
                            psy, ones_row[0:1, :], bp_sb[0:1, sl],
                            start=False, stop=True,
                        )
                    if j not in ysb_tiles:
                        ysb_tiles[j] = work.tile(
                            [128, 4, D], bf16, tag="ysb", name="ysb", bufs=2
                        )
                    ysb = ysb_tiles[j]
                    if j == NJ - 1:
                        # tail: ScalarE is idle after the last exp; store each
                        # half as soon as it lands
                        if lq >= 2:
                            nc.scalar.activation(
                                ysb[:, lq, sl], psy,
                                mybir.ActivationFunctionType.Copy,
                            )
                        else:
                            nc.vector.tensor_copy(out=ysb[:, lq, sl], in_=psy)
                        eng = nc.sync if (lq + half) % 2 == 0 else nc.scalar
                        eng.dma_start(
                            out=y_d[lc * 128:(lc + 1) * 128, sl], in_=ysb[:, lq, sl]
                        )
                    else:
                        nc.vector.tensor_copy(out=ysb[:, lq, sl], in_=psy)
                        if lq == 3 and half == 1:
                            # one batched store for the whole row-block group
                            nc.sync.dma_start(
                                out=y_d[4 * j * 128:(4 * j + 4) * 128, :].rearrange(
                                    "(q p) c -> p q c", q=4
                                ),
                                in_=ysb,
                            )

                units = []  # (cost_ns, fn)
                QK_NS, V_NS, PJ_NS = 1707.0, 853.0, 427.0
                prefix = {}
                for i in range(NJ):
                    units.append((QK_NS, (lambda m=0, i=i: emit_qk_unit(m, i))))
                    units.append((QK_NS, (lambda m=2, i=i: emit_qk_unit(m, i))))
                    if i == 0:
                        # V0-3 are injected inside att(0,0) so exp starts sooner
                        prefix[(0, 0)] = len(units)
                    else:
                        for lc in range(4 * i, 4 * i + 4):
                            units.append((V_NS, (lambda lc=lc: emit_v_unit(lc))))
                        prefix[(i, 0)] = len(units)
                    units.append((QK_NS, (lambda m=1, i=i: emit_qk_unit(m, i))))
                    units.append((QK_NS, (lambda m=3, i=i: emit_qk_unit(m, i))))
                    prefix[(i, 2)] = len(units)

                state = {"ptr": 0, "debt": 0.0}

                def drain_until(idx):
                    while state["ptr"] < idx:
                        cost, fn = units[state["ptr"]]
                        state["ptr"] += 1
                        fn()

                def drain_ns(budget):
                    state["debt"] += budget
                    while state["ptr"] < len(units) and state["debt"] > 0:
                        cost, fn = units[state["ptr"]]
                        state["ptr"] += 1
                        state["debt"] -= cost
                        fn()

                # ---- attention ----
                def emit_att_pair(j, h0, inject=()):
                    pl = h0 // 2
                    pos = [0, 64]
                    heads = [h0, h0 + 1]
                    outTs = [
                        ps_out.tile([KHD + 1, 512], f32, tag="outT", name="outT")
                        for _ in range(2)
                    ]
                    qrs = slice(j * 512, (j + 1) * 512)
                    last_c = 4 * j + 3
                    rounds = [("below", cp) for cp in range(0, 4 * j, 2)]
                    rounds += [("diag", 0), ("diag", 2)]
                    pending = []
                    inject = list(inject)

                    def flush_pending():
                        if inject and pending:
                            need = max(c for _, parts, _ in pending for c, _, _ in parts) + 1
                            while inject and inject[0][0] < need:
                                _, fn = inject.pop(0)
                                fn()
                        for hh, parts, ex in pending:
                            for c, exsl, n0 in parts:
                                nc.tensor.matmul(
                                    outTs[hh][:, n0:512],
                                    vaug[:, c, heads[hh], :],
                                    ex[:, exsl],
                                    start=(c == 0),
                                    stop=(c == last_c),
                                )
                        pending.clear()

                    for kind, arg in rounds:
                        new_pending = []
                        for hh in range(2):
                            po = pos[hh]
                            sc = ps_sc.tile([128, 1024], f32, tag="sc", name="sc")
                            if kind == "below":
                                cp = arg
                                for half in range(2):
                                    c = cp + half
                                    nc.tensor.matmul(
                                        sc[:, half * 512:(half + 1) * 512],
                                        kT_p[pl][po:po + 64, c * 128:(c + 1) * 128],
                                        qT_p[pl][po:po + 64, qrs],
                                        start=True,
                                        stop=True,
                                    )
                                ex = work.tile([128, 1024], bf16, tag="expT", name="ex", bufs=6)
                                nc.scalar.activation(
                                    ex, sc,
                                    mybir.ActivationFunctionType.Exp, scale=SCALE,
                                )
                                parts = [
                                    (cp, slice(0, 512), 0),
                                    (cp + 1, slice(512, 1024), 0),
                                ]
                            else:
                                i0 = arg
                                ws = [512 - 128 * (i0 + di) for di in range(2)]
                                offs = [0, ws[0]]
                                wtot = ws[0] + ws[1]
                                for di in range(2):
                                    c = 4 * j + i0 + di
                                    n0 = 128 * (i0 + di)
                                    nc.tensor.matmul(
                                        sc[:, offs[di]:offs[di] + ws[di]],
                                        kT_p[pl][po:po + 64, c * 128:(c + 1) * 128],
                                        qT_p[pl][po:po + 64, j * 512 + n0:(j + 1) * 512],
                                        start=True,
                                        stop=True,
                                    )
                                ex = work.tile([128, 1024], bf16, tag="expT", name="ex", bufs=6)
                                nc.scalar.activation(
                                    ex[:, 0:wtot], sc[:, 0:wtot],
                                    mybir.ActivationFunctionType.Exp, scale=SCALE,
                                )
                                for di in range(2):
                                    nc.vector.tensor_mul(
                                        ex[:, offs[di]:offs[di] + 128],
                                        ex[:, offs[di]:offs[di] + 128],
                                        tri_sb,
                                    )
                                parts = [
                                    (4 * j + i0, slice(0, ws[0]), 128 * i0),
                                    (
                                        4 * j + i0 + 1,
                                        slice(offs[1], offs[1] + ws[1]),
                                        128 * (i0 + 1),
                                    ),
                                ]
                            new_pending.append((hh, parts, ex))
                        # filler between the scores and the exp-gated PV flush
                        if kind == "below":
                            drain_ns(800.0)
                        elif arg == 0:
                            drain_ns(650.0)
                        else:
                            drain_ns(120.0)
                        flush_pending()
                        pending.extend(new_pending)
                    flush_pending()
                    # normalize: attnT[f, qrow] = outT[f, qrow] / outT[64, qrow]
                    for hh in range(2):
                        po = pos[hh]
                        recip = small.tile([1, 512], bf16, tag="recip", name="recip")
                        nc.vector.reciprocal(recip, outTs[hh][KHD:KHD + 1, :])
                        bc_sb = small.tile([64, 512], bf16, tag="bcsb", name="bc_sb")
                        nc.gpsimd.partition_broadcast(bc_sb, recip)
                        if j == NJ - 1:
                            # chunked so the tail projection can start per-slice
                            for q4 in range(4):
                                qs = slice(q4 * 128, (q4 + 1) * 128)
                                nc.vector.tensor_mul(
                                    attnT_js[j][po:po + 64, pl, qs],
                                    outTs[hh][0:KHD, qs],
                                    bc_sb[:, qs],
                                )
                        else:
                            nc.vector.tensor_mul(
                                attnT_js[j][po:po + 64, pl, :], outTs[hh][0:KHD, :], bc_sb
                            )

                # ---- emission schedule ----
                for j in range(NJ):
                    for h0 in (0, 2):
                        drain_until(prefix[(j, h0)])
                        inj = ()
                        if j == 0 and h0 == 0:
                            inj = [(lc, (lambda lc=lc: emit_v_unit(lc)))
                                   for lc in range(4)]
                        emit_att_pair(j, h0, inj)
                    for lq in range(4):
                        for half in range(2):
                            units.append(
                                (PJ_NS, (lambda j=j, lq=lq, half=half:
                                         emit_proj_unit(j, lq, half)))
                            )
                drain_until(len(units))

    nc.compile()
    return nc


def kernel(input_BLD, W_attn, b_attn, W_proj, b_proj):
    import ml_dtypes

    bf16 = ml_dtypes.bfloat16
    input_BLD = np.asarray(input_BLD, dtype=np.float32)
    W_attn = np.asarray(W_attn, dtype=np.float32)
    b_attn = np.asarray(b_attn, dtype=np.float32)
    W_proj = np.asarray(W_proj, dtype=np.float32)
    b_proj = np.asarray(b_proj, dtype=np.float32)

    has_bqk = bool(np.any(b_attn[: 2 * D]))
    has_bv = bool(np.any(b_attn[2 * D:]))
    has_bp = bool(np.any(b_proj))

    key = (has_bqk, has_bv, has_bp)
    if key not in _CACHE:
        _CACHE[key] = _build(*key)
    nc = _CACHE[key]

    tri = (np.arange(128)[None, :] >= np.arange(128)[:, None]).astype(bf16)
    in_maps = []
    for c in range(NCORES):
        b, t = divmod(c, 4)
        hs = t * HPC * KHD  # feature offset of this core's heads
        w_loc = np.concatenate(
            [
                W_attn[hs:hs + FV],  # q rows
                W_attn[D + hs:D + hs + FV],  # k rows
                W_attn[2 * D + hs:2 * D + hs + FV],  # v rows
            ],
            axis=0,
        )  # [768, 1024]
        m = {
            "xT": np.ascontiguousarray(input_BLD[b].T).astype(bf16),
            "wqkvT": np.ascontiguousarray(w_loc.T).astype(bf16),
            "wpT": np.ascontiguousarray(W_proj[:, hs:hs + FV].T).astype(bf16),
            "tri": tri,
        }
        if has_bv or has_bp:
            m["onesrow"] = np.ones((1, 128), bf16)
        if has_bqk:
            bqk = np.concatenate([b_attn[hs:hs + FV], b_attn[D + hs:D + hs + FV]])
            m["bqk"] = np.ascontiguousarray(bqk.reshape(FQK // 128, 128).T)
        if has_bv:
            m["bv"] = b_attn[2 * D + hs:2 * D + hs + FV][None, :].astype(bf16)
        if has_bp:
            m["bp"] = (b_proj / 4.0)[None, :].astype(bf16)
        in_maps.append(m)

    from concourse.bass_utils import run_bass_kernel_spmd

    globals()["_last_in_maps"] = in_maps
    res = run_bass_kernel_spmd(nc, in_maps, list(range(NCORES)))
    globals()["_LAST_RESULTS"] = res
    out = np.empty((B, L, D), dtype=np.float32)
    for b in range(B):
        acc = res.results[4 * b]["y"].astype(np.float32)
        for t in range(1, 4):
            acc = acc + res.results[4 * b + t]["y"].astype(np.float32)
        out[b] = acc
    return out
